# revision 24
# baseline (speedup 1.0000x reference)
"""DiffJPEG forward (DCT -> quantize(round) -> dequant -> IDCT on 8x8 blocks)
as a Bass/Tile kernel on 8 Trainium2 NeuronCores, pure data parallel over the
batch dim.

Input : img_orig (32, 3, 512, 512) f32, full (unsharded).
Output: (32, 3, 512, 512) f32.

Per-core shard: 4 batches x 3 channels = 12 images of 512x512.

Per image pipeline (all layouts [partition, free]):
  A: fused H-DCT + transpose: for each (s,c) 128x128 chunk of the image,
     matmul(lhsT=X_chunk[h,w], rhs=KDT[h,i]) -> T1_c[w, i]   (PSUM)
  B: W-DCT: matmul(lhsT=KDT, rhs=T1_c) -> Z_c[j, i]          (PSUM)
  Q: scaled = Z*invQ ; round via +/- 1.5*2^23 ; dequant *Q   (DVE, SBUF)
  C: fused W-IDCT + transpose: matmul(lhsT=Rq_c[j, i-chunk], rhs=KD[j,w])
     -> V_s[i, w]                                            (PSUM)
  D: H-IDCT: matmul(lhsT=KD, rhs=V_s) -> F_s[h, w] -> DMA out
where KD = kron(I16, D) and KDT = KD.T, D the 8x8 DCT matrix.
"""

import os
import sys

if "/opt/trn_rl_repo" not in sys.path:
    sys.path.insert(0, "/opt/trn_rl_repo")

import numpy as np

N_CORES = 8
B, C, H, W = 32, 3, 512, 512
IMGS_PER_CORE = (B // N_CORES) * C  # 12
ROWS = IMGS_PER_CORE * H  # 6144

MAGIC = float(np.float32(1.5 * 2**23))

# Matmul operand dtypes (tunable): "float32" exact, "float32r" fast.
MM_DT = os.environ.get("KMM_DT", "float32r")
AC_DT = os.environ.get("KAC_DT", "bfloat16")


def _dct_matrix(n=8):
    i = np.arange(n)[:, None]
    j = np.arange(n)[None, :]
    c = np.where(i == 0, np.sqrt(1.0 / n), np.sqrt(2.0 / n))
    return (c * np.cos((2 * j + 1) * i * np.pi / (2 * n))).astype(np.float32)


def _q_matrix(quality=50):
    q_luma = np.array(
        [
            [16, 11, 10, 16, 24, 40, 51, 61],
            [12, 12, 14, 19, 26, 58, 60, 55],
            [14, 13, 16, 24, 40, 57, 69, 56],
            [14, 17, 22, 29, 51, 87, 80, 62],
            [18, 22, 37, 56, 68, 109, 103, 77],
            [24, 35, 55, 64, 81, 104, 113, 92],
            [49, 64, 78, 87, 103, 121, 120, 101],
            [72, 92, 95, 98, 112, 100, 103, 99],
        ],
        dtype=np.float32,
    )
    scale = 5000.0 / quality if quality < 50 else 200.0 - quality * 2.0
    return np.maximum(np.floor((q_luma * scale + 50.0) / 100.0), 1.0).astype(
        np.float32
    )


def make_constants():
    D = _dct_matrix()
    Q = _q_matrix(50)
    KD = np.kron(np.eye(16, dtype=np.float32), D)  # [128,128]
    KDT = np.ascontiguousarray(KD.T)
    ii, jj = np.meshgrid(np.arange(512) % 8, np.arange(128) % 8, indexing="xy")
    # pat[p, f] = Q[f%8, p%8]
    Qpat = Q[ii, jj].astype(np.float32)
    invQpat = (np.float32(1.0) / Qpat).astype(np.float32)
    return KD, KDT, Qpat, invQpat


_COMPILED = None


def _build():
    global _COMPILED
    if _COMPILED is not None:
        return _COMPILED

    import concourse.mybir as mybir
    import concourse.bacc as bacc
    import concourse.tile as tile

    dt = mybir.dt
    mm_dt = getattr(dt, MM_DT)

    ac_dt = dt.bfloat16 if AC_DT == "bfloat16" else getattr(dt, AC_DT)

    nc = bacc.Bacc("TRN2", target_bir_lowering=False, debug=False,
                   num_devices=N_CORES)

    x_dram = nc.dram_tensor("x", [ROWS, W], dt.float32, kind="ExternalInput").ap()
    y_dram = nc.dram_tensor("y", [ROWS, W], dt.float32, kind="ExternalOutput").ap()
    kd_dram = nc.dram_tensor("kd", [128, 128], dt.float32, kind="ExternalInput").ap()
    kdt_dram = nc.dram_tensor("kdt", [128, 128], dt.float32, kind="ExternalInput").ap()
    qpat_dram = nc.dram_tensor("qpat", [128, 512], dt.float32, kind="ExternalInput").ap()
    iqpat_dram = nc.dram_tensor("iqpat", [128, 512], dt.float32, kind="ExternalInput").ap()

    with tile.TileContext(nc) as tc:
        with (
            tc.tile_pool(name="const", bufs=1) as cpool,
            tc.tile_pool(name="xin", bufs=4) as xpool,
            tc.tile_pool(name="t1", bufs=10) as t1pool,
            tc.tile_pool(name="rq", bufs=14) as rqpool,
            tc.tile_pool(name="scr", bufs=8) as scrpool,
            tc.tile_pool(name="u", bufs=8) as upool,
            tc.tile_pool(name="pa", bufs=2, space="PSUM") as papool,
            tc.tile_pool(name="pb", bufs=2, space="PSUM") as pbpool,
            tc.tile_pool(name="pc", bufs=2, space="PSUM") as pcpool,
            tc.tile_pool(name="pd", bufs=2, space="PSUM") as pdpool,
        ):
            kd = cpool.tile([128, 128], dt.float32, tag="kd")
            kdt = cpool.tile([128, 128], dt.float32, tag="kdt")
            qpat = cpool.tile([128, 512], dt.float32, tag="qpat")
            iqpat = cpool.tile([128, 512], dt.float32, tag="iqpat")
            nc.sync.dma_start(kd[:], kd_dram[:])
            nc.sync.dma_start(kdt[:], kdt_dram[:])
            nc.sync.dma_start(qpat[:], qpat_dram[:])
            nc.sync.dma_start(iqpat[:], iqpat_dram[:])
            qpatb = cpool.tile([128, 1024], ac_dt, tag="qpatb")
            nc.vector.tensor_copy(qpatb[:, :512], qpat[:])
            nc.vector.tensor_copy(qpatb[:, 512:], qpat[:])
            iqpat2 = cpool.tile([128, 1024], dt.float32, tag="iqpat2")
            nc.vector.tensor_copy(iqpat2[:, :512], iqpat[:])
            nc.vector.tensor_copy(iqpat2[:, 512:], iqpat[:])
            kdb = cpool.tile([128, 128], ac_dt, tag="kdb")
            kdtb = cpool.tile([128, 128], ac_dt, tag="kdtb")
            nc.vector.tensor_copy(kdb[:], kd[:])
            nc.vector.tensor_copy(kdtb[:], kdt[:])

            def phase_ab(im):
                # ---- load image (bf16 cast in DMA): X[p, 512*s + w] = img[128s+p, w]
                x = xpool.tile([128, 2048], ac_dt, tag="x", name=f"x{im}")
                src_ap = x_dram[im * 512:(im + 1) * 512, :]
                nc.gpsimd.dma_start(
                    x[:].rearrange("p (s w) -> p s w", s=4),
                    src_ap.rearrange("(s p) w -> p s w", p=128),
                )
                rqs = []
                for c in range(4):
                    # ---- phase A: fused H-DCT + transpose
                    ta = papool.tile([128, 512], dt.float32, tag="pa", name=f"ta{im}_{c}")
                    for s in range(4):
                        nc.tensor.matmul(
                            ta[:, 128 * s:128 * (s + 1)],
                            lhsT=x[:, 512 * s + 128 * c: 512 * s + 128 * (c + 1)],
                            rhs=kdtb[:],
                        )
                    t1 = t1pool.tile([128, 512], ac_dt, tag="t1", name=f"t1_{im}_{c}")
                    nc.scalar.copy(t1[:], ta[:])

                    # ---- phase B: W-DCT
                    zb = pbpool.tile([128, 512], dt.float32, tag="pb", name=f"zb{im}_{c}")
                    nc.tensor.matmul(zb[:], lhsT=kdtb[:], rhs=t1[:])

                    # ---- quantize
                    s_t = scrpool.tile([128, 512], ac_dt, tag="s", name=f"s{im}_{c}")
                    nc.vector.tensor_tensor(
                        s_t[:], zb[:], iqpat[:], mybir.AluOpType.mult
                    )
                    r_t = scrpool.tile([128, 512], ac_dt, tag="r", name=f"r{im}_{c}")
                    nc.vector.tensor_scalar(
                        r_t[:], s_t[:], MAGIC, MAGIC,
                        mybir.AluOpType.add, mybir.AluOpType.subtract,
                    )
                    rq = rqpool.tile([128, 512], ac_dt, tag="rq", name=f"rq{im}_{c}")
                    nc.vector.tensor_tensor(
                        rq[:], r_t[:], qpatb[:, :512], mybir.AluOpType.mult
                    )
                    rqs.append(rq)
                return rqs

            def phase_cd(im, rqs):
                for s in range(4):
                    # ---- phase C: fused W-IDCT + transpose
                    vc = pcpool.tile([128, 512], dt.float32, tag="pc", name=f"vc{im}_{s}")
                    for c in range(4):
                        nc.tensor.matmul(
                            vc[:, 128 * c:128 * (c + 1)],
                            lhsT=rqs[c][:, 128 * s:128 * (s + 1)],
                            rhs=kdb[:],
                        )
                    u = upool.tile([128, 512], ac_dt, tag="u", name=f"u{im}_{s}")
                    nc.scalar.copy(u[:], vc[:])

                    # ---- phase D: H-IDCT
                    fd = pdpool.tile([128, 512], dt.float32, tag="pd", name=f"fd{im}_{s}")
                    nc.tensor.matmul(fd[:], lhsT=kdb[:], rhs=u[:])
                    fs = upool.tile([128, 512], dt.float32, tag="f", name=f"fs{im}_{s}")
                    if s % 2 == 0:
                        nc.scalar.copy(fs[:], fd[:])
                    else:
                        nc.vector.tensor_copy(fs[:], fd[:])

                    # ---- store
                    nc.sync.dma_start(
                        y_dram[im * 512 + 128 * s: im * 512 + 128 * (s + 1), :],
                        fs[:],
                    )

            pending = []
            for im in range(IMGS_PER_CORE):
                pending.append((im, phase_ab(im)))
                if len(pending) > 2:
                    pim, prqs = pending.pop(0)
                    phase_cd(pim, prqs)
            for pim, prqs in pending:
                phase_cd(pim, prqs)

    nc.compile()
    _COMPILED = nc
    return nc


def kernel(img_orig: np.ndarray, _trace: bool = False):
    from concourse import bass_utils

    nc = _build()

    img = np.ascontiguousarray(img_orig, dtype=np.float32)
    assert img.shape == (B, C, H, W)

    KD, KDT, Qpat, invQpat = make_constants()
    shards = img.reshape(N_CORES, IMGS_PER_CORE * H, W)

    in_maps = [
        {
            "x": shards[i],
            "kd": KD,
            "kdt": KDT,
            "qpat": Qpat,
            "iqpat": invQpat,
        }
        for i in range(N_CORES)
    ]

    res = bass_utils.run_bass_kernel_spmd(
        nc, in_maps, core_ids=list(range(N_CORES)), trace=_trace
    )

    out = np.empty((N_CORES, IMGS_PER_CORE * H, W), dtype=np.float32)
    for i in range(N_CORES):
        out[i] = res.results[i]["y"]
    result = out.reshape(B, C, H, W)
    if _trace:
        return result, res
    return result


# revision 25
# speedup vs baseline: 1.0565x; 1.0565x over previous
"""DiffJPEG forward (DCT -> quantize(round) -> dequant -> IDCT on 8x8 blocks)
as a Bass/Tile kernel on 8 Trainium2 NeuronCores, pure data parallel over the
batch dim.

Input : img_orig (32, 3, 512, 512) f32, full (unsharded).
Output: (32, 3, 512, 512) f32.

Per-core shard: 4 batches x 3 channels = 12 images of 512x512.

Per image pipeline (all layouts [partition, free]):
  A: fused H-DCT + transpose: for each (s,c) 128x128 chunk of the image,
     matmul(lhsT=X_chunk[h,w], rhs=KDT[h,i]) -> T1_c[w, i]   (PSUM)
  B: W-DCT: matmul(lhsT=KDT, rhs=T1_c) -> Z_c[j, i]          (PSUM)
  Q: scaled = Z*invQ ; round via +/- 1.5*2^23 ; dequant *Q   (DVE, SBUF)
  C: fused W-IDCT + transpose: matmul(lhsT=Rq_c[j, i-chunk], rhs=KD[j,w])
     -> V_s[i, w]                                            (PSUM)
  D: H-IDCT: matmul(lhsT=KD, rhs=V_s) -> F_s[h, w] -> DMA out
where KD = kron(I16, D) and KDT = KD.T, D the 8x8 DCT matrix.
"""

import os
import sys

if "/opt/trn_rl_repo" not in sys.path:
    sys.path.insert(0, "/opt/trn_rl_repo")

import numpy as np

N_CORES = 8
B, C, H, W = 32, 3, 512, 512
IMGS_PER_CORE = (B // N_CORES) * C  # 12
ROWS = IMGS_PER_CORE * H  # 6144

MAGIC = float(np.float32(1.5 * 2**23))

# Matmul operand dtypes (tunable): "float32" exact, "float32r" fast.
MM_DT = os.environ.get("KMM_DT", "float32r")
AC_DT = os.environ.get("KAC_DT", "bfloat16")


def _dct_matrix(n=8):
    i = np.arange(n)[:, None]
    j = np.arange(n)[None, :]
    c = np.where(i == 0, np.sqrt(1.0 / n), np.sqrt(2.0 / n))
    return (c * np.cos((2 * j + 1) * i * np.pi / (2 * n))).astype(np.float32)


def _q_matrix(quality=50):
    q_luma = np.array(
        [
            [16, 11, 10, 16, 24, 40, 51, 61],
            [12, 12, 14, 19, 26, 58, 60, 55],
            [14, 13, 16, 24, 40, 57, 69, 56],
            [14, 17, 22, 29, 51, 87, 80, 62],
            [18, 22, 37, 56, 68, 109, 103, 77],
            [24, 35, 55, 64, 81, 104, 113, 92],
            [49, 64, 78, 87, 103, 121, 120, 101],
            [72, 92, 95, 98, 112, 100, 103, 99],
        ],
        dtype=np.float32,
    )
    scale = 5000.0 / quality if quality < 50 else 200.0 - quality * 2.0
    return np.maximum(np.floor((q_luma * scale + 50.0) / 100.0), 1.0).astype(
        np.float32
    )


def make_constants():
    D = _dct_matrix()
    Q = _q_matrix(50)
    KD = np.kron(np.eye(16, dtype=np.float32), D)  # [128,128]
    KDT = np.ascontiguousarray(KD.T)
    ii, jj = np.meshgrid(np.arange(512) % 8, np.arange(128) % 8, indexing="xy")
    # pat[p, f] = Q[f%8, p%8]
    Qpat = Q[ii, jj].astype(np.float32)
    invQpat = (np.float32(1.0) / Qpat).astype(np.float32)
    return KD, KDT, Qpat, invQpat


_COMPILED = None


def _build():
    global _COMPILED
    if _COMPILED is not None:
        return _COMPILED

    import concourse.mybir as mybir
    import concourse.bacc as bacc
    import concourse.tile as tile

    dt = mybir.dt
    mm_dt = getattr(dt, MM_DT)

    ac_dt = dt.bfloat16 if AC_DT == "bfloat16" else getattr(dt, AC_DT)

    nc = bacc.Bacc("TRN2", target_bir_lowering=False, debug=False,
                   num_devices=N_CORES)

    x_dram = nc.dram_tensor("x", [ROWS, W], dt.float32, kind="ExternalInput").ap()
    y_dram = nc.dram_tensor("y", [ROWS, W], dt.float32, kind="ExternalOutput").ap()
    kd_dram = nc.dram_tensor("kd", [128, 128], dt.float32, kind="ExternalInput").ap()
    kdt_dram = nc.dram_tensor("kdt", [128, 128], dt.float32, kind="ExternalInput").ap()
    qpat_dram = nc.dram_tensor("qpat", [128, 512], dt.float32, kind="ExternalInput").ap()
    iqpat_dram = nc.dram_tensor("iqpat", [128, 512], dt.float32, kind="ExternalInput").ap()

    with tile.TileContext(nc) as tc:
        with (
            tc.tile_pool(name="const", bufs=1) as cpool,
            tc.tile_pool(name="xin", bufs=4) as xpool,
            tc.tile_pool(name="t1", bufs=10) as t1pool,
            tc.tile_pool(name="rq", bufs=14) as rqpool,
            tc.tile_pool(name="scr", bufs=8) as scrpool,
            tc.tile_pool(name="u", bufs=8) as upool,
            tc.tile_pool(name="pa", bufs=2, space="PSUM") as papool,
            tc.tile_pool(name="pb", bufs=2, space="PSUM") as pbpool,
            tc.tile_pool(name="pc", bufs=2, space="PSUM") as pcpool,
            tc.tile_pool(name="pd", bufs=2, space="PSUM") as pdpool,
        ):
            kd = cpool.tile([128, 128], dt.float32, tag="kd")
            kdt = cpool.tile([128, 128], dt.float32, tag="kdt")
            qpat = cpool.tile([128, 512], dt.float32, tag="qpat")
            iqpat = cpool.tile([128, 512], dt.float32, tag="iqpat")
            nc.sync.dma_start(kd[:], kd_dram[:])
            nc.sync.dma_start(kdt[:], kdt_dram[:])
            nc.sync.dma_start(qpat[:], qpat_dram[:])
            nc.sync.dma_start(iqpat[:], iqpat_dram[:])
            qpatb = cpool.tile([128, 1024], ac_dt, tag="qpatb")
            nc.vector.tensor_copy(qpatb[:, :512], qpat[:])
            nc.vector.tensor_copy(qpatb[:, 512:], qpat[:])
            iqpat2 = cpool.tile([128, 1024], dt.float32, tag="iqpat2")
            nc.vector.tensor_copy(iqpat2[:, :512], iqpat[:])
            nc.vector.tensor_copy(iqpat2[:, 512:], iqpat[:])
            kdb = cpool.tile([128, 128], ac_dt, tag="kdb")
            kdtb = cpool.tile([128, 128], ac_dt, tag="kdtb")
            nc.vector.tensor_copy(kdb[:], kd[:])
            nc.vector.tensor_copy(kdtb[:], kdt[:])

            def phase_ab(im):
                # ---- load image (bf16 cast in DMA): X[p, 512*s + w] = img[128s+p, w]
                x = xpool.tile([128, 2048], ac_dt, tag="x", name=f"x{im}")
                src_ap = x_dram[im * 512:(im + 1) * 512, :]
                nc.gpsimd.dma_start(
                    x[:].rearrange("p (s w) -> p s w", s=4),
                    src_ap.rearrange("(s p) w -> p s w", p=128),
                )
                rqs = []
                for c in range(4):
                    # ---- phase A: fused H-DCT + transpose
                    ta = papool.tile([128, 512], dt.float32, tag="pa", name=f"ta{im}_{c}")
                    for s in range(4):
                        nc.tensor.matmul(
                            ta[:, 128 * s:128 * (s + 1)],
                            lhsT=x[:, 512 * s + 128 * c: 512 * s + 128 * (c + 1)],
                            rhs=kdtb[:],
                        )
                    t1 = t1pool.tile([128, 512], ac_dt, tag="t1", name=f"t1_{im}_{c}")
                    nc.scalar.copy(t1[:], ta[:])

                    # ---- phase B: W-DCT
                    zb = pbpool.tile([128, 512], dt.float32, tag="pb", name=f"zb{im}_{c}")
                    nc.tensor.matmul(zb[:], lhsT=kdtb[:], rhs=t1[:])

                    # ---- quantize
                    s_t = scrpool.tile([128, 512], ac_dt, tag="s", name=f"s{im}_{c}")
                    nc.vector.tensor_tensor(
                        s_t[:], zb[:], iqpat[:], mybir.AluOpType.mult
                    )
                    r_t = scrpool.tile([128, 512], ac_dt, tag="r", name=f"r{im}_{c}")
                    nc.vector.tensor_scalar(
                        r_t[:], s_t[:], MAGIC, MAGIC,
                        mybir.AluOpType.add, mybir.AluOpType.subtract,
                    )
                    rq = rqpool.tile([128, 512], ac_dt, tag="rq", name=f"rq{im}_{c}")
                    nc.vector.tensor_tensor(
                        rq[:], r_t[:], qpatb[:, :512], mybir.AluOpType.mult
                    )
                    rqs.append(rq)
                return rqs

            def phase_cd(im, rqs):
                for s in range(4):
                    # ---- phase C: fused W-IDCT + transpose
                    vc = pcpool.tile([128, 512], dt.float32, tag="pc", name=f"vc{im}_{s}")
                    for c in range(4):
                        nc.tensor.matmul(
                            vc[:, 128 * c:128 * (c + 1)],
                            lhsT=rqs[c][:, 128 * s:128 * (s + 1)],
                            rhs=kdb[:],
                        )
                    u = upool.tile([128, 512], ac_dt, tag="u", name=f"u{im}_{s}")
                    nc.scalar.copy(u[:], vc[:])

                    # ---- phase D: H-IDCT
                    fd = pdpool.tile([128, 512], dt.float32, tag="pd", name=f"fd{im}_{s}")
                    nc.tensor.matmul(fd[:], lhsT=kdb[:], rhs=u[:])
                    fs = upool.tile([128, 512], dt.float32, tag="f", name=f"fs{im}_{s}")
                    if s % 2 == 0:
                        nc.scalar.copy(fs[:], fd[:])
                    else:
                        nc.vector.tensor_copy(fs[:], fd[:])

                    # ---- store
                    nc.sync.dma_start(
                        y_dram[im * 512 + 128 * s: im * 512 + 128 * (s + 1), :],
                        fs[:],
                    )

            pending = []
            for im in range(IMGS_PER_CORE):
                if len(pending) >= 2:
                    pim, prqs = pending.pop(0)
                    phase_cd(pim, prqs)
                pending.append((im, phase_ab(im)))
            for pim, prqs in pending:
                phase_cd(pim, prqs)

    nc.compile()
    _COMPILED = nc
    return nc


def kernel(img_orig: np.ndarray, _trace: bool = False):
    from concourse import bass_utils

    nc = _build()

    img = np.ascontiguousarray(img_orig, dtype=np.float32)
    assert img.shape == (B, C, H, W)

    KD, KDT, Qpat, invQpat = make_constants()
    shards = img.reshape(N_CORES, IMGS_PER_CORE * H, W)

    in_maps = [
        {
            "x": shards[i],
            "kd": KD,
            "kdt": KDT,
            "qpat": Qpat,
            "iqpat": invQpat,
        }
        for i in range(N_CORES)
    ]

    res = bass_utils.run_bass_kernel_spmd(
        nc, in_maps, core_ids=list(range(N_CORES)), trace=_trace
    )

    out = np.empty((N_CORES, IMGS_PER_CORE * H, W), dtype=np.float32)
    for i in range(N_CORES):
        out[i] = res.results[i]["y"]
    result = out.reshape(B, C, H, W)
    if _trace:
        return result, res
    return result
